# revision 47
# baseline (speedup 1.0000x reference)
"""Trainium2 Bass kernel for nn_Attention_54614804136573 (topk_masking).

Sharding: 8 cores = 4 batches x 2 head-groups (8 heads each). Each core gets
its batch's x pre-transposed on host to bf16 [d, n] (rows rotated so its own
8 head-chunks come first), computes the token-importance mask redundantly,
runs its 8 heads of attention, and produces a partial to_out product for all
2048 output channels. The host sums the two partials per batch and adds bo.

Device-side structure (v4):
  - xT chunks are DMA'd directly in bf16; Q/K/V projections + logits consume
    each chunk as it lands. V is transposed to its natural layout with DMA
    transposes (no PE/PSUM involvement) and masked in place on gpsimd.
  - the token mask is binary: the bottom-25 softmax values are ~1e-3 and are
    snapped to 0 (their exp contribution becomes exactly 1 via a 0 scale,
    their V rows 0), which is well inside the error budget.
  - softmax denominator off the PE: pairwise bf16 tree-adds of the exp tiles
    on DVE (in place in the pexp tile), then a gpsimd partition_all_reduce
    (f32 accum, broadcast), reciprocal + fused normalize on DVE.
  - to_out accumulates per output-chunk over head-chunks 1..7 (h-outer so
    consecutive matmuls share stationary weights) and finishes with head 0,
    whose outT is ready first.
"""

import sys

sys.path.insert(0, "/opt/trn_rl_repo")

import numpy as np
import ml_dtypes

import concourse.mybir as mybir
import concourse.tile as tile
from concourse import bacc, bass_utils
from concourse import bass_isa

B = 4
N = 1024
C = 128
D = 2048
NCHUNK = 16  # d-chunks of 128 (= patch positions = heads)
HPC = 8  # heads per core
MASK_NUM = 25
SCALE = 64.0 ** -0.5  # 0.125

F32 = mybir.dt.float32
BF16 = mybir.dt.bfloat16
U32 = mybir.dt.uint32
Exp = mybir.ActivationFunctionType.Exp
Ident = mybir.ActivationFunctionType.Identity
Copy = mybir.ActivationFunctionType.Copy
NEG_BIG = -1e30
Add = mybir.AluOpType.add
Mult = mybir.AluOpType.mult


def _body(tc, xTd, wq_d, wk_d, wv_d, bq_d, bk_d, bv_d, wtc_d, wo_d, outT_d):
    nc = tc.nc

    with (
        tc.tile_pool(name="consts", bufs=1) as consts,
        tc.tile_pool(name="persist", bufs=1) as persist,
    ):
        # ---- persistent activations ----
        qT = persist.tile([128, HPC, N], BF16)  # [c', h, n] 2 MB
        kT = persist.tile([128, HPC, N], BF16)  # 2 MB
        vnat = persist.tile([128, HPC, 8, C], BF16)  # [j, h, jt, c] 2 MB
        mrow_b = persist.tile([128, N], F32)  # binary mask broadcast to all p
        woT_sb = persist.tile([128, HPC, D], BF16)  # [d, h-chunk, o] 4 MB

        # ================= phase 1: stream xT chunks, QKV, logits, mask ====
        xT_own = persist.tile([128, HPC, N], BF16)  # [c, k, n] own half 2 MB

        with (
            tc.tile_pool(name="ph1big", bufs=1) as ph1big,
            tc.tile_pool(name="mrows", bufs=1) as mrows,
            tc.tile_pool(name="mm_psum", bufs=3, space="PSUM") as mm_psum,
            tc.tile_pool(name="lg_psum", bufs=1, space="PSUM") as lg_psum,
        ):
            xT_p = ph1big.tile([128, HPC, N], BF16)  # partner half 2 MB

            # small weights first on the sync queue so projections can start
            # the moment chunk 0 lands
            wq_sb = consts.tile([C, C], BF16)
            nc.sync.dma_start(out=wq_sb, in_=wq_d)
            wk_sb = consts.tile([C, C], BF16)
            nc.sync.dma_start(out=wk_sb, in_=wk_d)
            wv_sb = consts.tile([C, C], BF16)
            nc.sync.dma_start(out=wv_sb, in_=wv_d)
            bq_sb = consts.tile([C, 1], F32)
            nc.sync.dma_start(out=bq_sb, in_=bq_d)
            bk_sb = consts.tile([C, 1], F32)
            nc.sync.dma_start(out=bk_sb, in_=bk_d)
            bv_sb = consts.tile([C, 1], F32)
            nc.sync.dma_start(out=bv_sb, in_=bv_d)
            wtc_sb = consts.tile([C, 1], BF16)
            nc.sync.dma_start(out=wtc_sb, in_=wtc_d)

            # own chunks 0..7 on sync; partner chunks (mask logits only) on
            # the gpsimd queue, keeping the Act sequencer DMA-free. Wo rides
            # the sync queue right after x: the serial DMA path is idle in
            # the window between the x chunks and the (post-mask) V
            # transposes.
            for k in range(HPC):
                nc.sync.dma_start(
                    out=xT_own[:, k, :], in_=xTd[k * 128 : (k + 1) * 128, :]
                )
            for k in range(HPC, NCHUNK):
                nc.gpsimd.dma_start(
                    out=xT_p[:, k - HPC, :],
                    in_=xTd[k * 128 : (k + 1) * 128, :],
                )
            for h in range(HPC):
                nc.sync.dma_start(
                    out=woT_sb[:, h, :], in_=wo_d[h * 128 : (h + 1) * 128, :]
                )

            # mask-row scratch, zero/one rows prepared off the critical path
            onesrow = mrows.tile([1, N], F32)
            nc.vector.memset(onesrow, 1.0)
            maskrow = mrows.tile([1, N], F32)
            nc.vector.memset(maskrow, 0.0)

            lg = lg_psum.tile([1, N], F32)

            # ALL logits matmuls first, in DMA-arrival order (own/partner
            # interleaved): the mask chain is the critical path and must not
            # be backpressured by the projection pipeline. Then Q/K/V
            # projections; V stays in [c', j] row layout until the mask
            # lands.
            arrival = [k for p in range(HPC) for k in (p, HPC + p)]
            for idx, k in enumerate(arrival):
                src = xT_own[:, k, :] if k < HPC else xT_p[:, k - HPC, :]
                for half in range(2):
                    nc.tensor.matmul(
                        lg[:, half * 512 : (half + 1) * 512],
                        wtc_sb,
                        src[:, half * 512 : (half + 1) * 512],
                        start=(idx == 0),
                        stop=(idx == NCHUNK - 1),
                    )
            # Q/K projections only — the V path (projection + fused
            # bias/mask + DMA-transpose) runs in the attention era where PE
            # and DVE have slack and the Act sequencer must stay clear for
            # the exps.
            for h in range(HPC):
                for w_sb, b_sb, dstT in ((wq_sb, bq_sb, qT), (wk_sb, bk_sb, kT)):
                    pp = mm_psum.tile([128, N], F32)
                    for half in range(2):
                        nc.tensor.matmul(
                            pp[:, half * 512 : (half + 1) * 512],
                            w_sb,
                            xT_own[:, h, half * 512 : (half + 1) * 512],
                            start=True,
                            stop=True,
                        )
                    nc.scalar.activation(
                        out=dstT[:, h, :], in_=pp, func=Ident, bias=b_sb
                    )

            # ---- binary mask: 1 for tokens above the 25th-smallest logit,
            # 0 for the bottom 25 (their softmax values are ~1e-3; dropping
            # them costs ~1e-3 relative error). Top-k scratch in bf16 for
            # DVE 2x throughput.
            scratch = mrows.tile([1, N], BF16)
            nc.vector.tensor_scalar_mul(scratch, lg, -1.0)
            m8 = mrows.tile([1, 8], BF16)
            for _ in range(3):
                nc.vector.max(out=m8, in_=scratch)
                nc.vector.match_replace(
                    out=scratch, in_to_replace=m8, in_values=scratch,
                    imm_value=NEG_BIG,
                )
            nc.vector.max(out=m8, in_=scratch)  # m8[0,0] = 25th largest of -L
            m8neg = mrows.tile([1, 1], F32)
            nc.vector.tensor_scalar_mul(m8neg, m8[:, 0:1], -1.0)
            sel = mrows.tile([1, N], U32)
            nc.vector.tensor_scalar(
                sel, lg, m8neg, None, op0=mybir.AluOpType.is_gt
            )
            nc.vector.copy_predicated(maskrow, sel, onesrow)
            # broadcast the mask row to all 128 partitions (no DRAM trip):
            # the mask is only ever needed along a FREE dim from here on
            nc.gpsimd.partition_broadcast(mrow_b, maskrow, channels=128)

        # ================= phase 2: attention + to_out =====================
        with tc.tile_pool(name="ph2big", bufs=1) as ph2big:
            outT_sb = ph2big.tile([128, HPC, N], BF16)  # [c, h, i] 2 MB

            attn_pools = (
                tc.tile_pool(name="pexp", bufs=2),
                tc.tile_pool(name="dvp", bufs=2),
                tc.tile_pool(name="kts", bufs=2),
                tc.tile_pool(name="vtm", bufs=2),
                tc.tile_pool(name="dsum", bufs=8),
                tc.tile_pool(name="fout", bufs=3),
                tc.tile_pool(name="st_psum", bufs=2, space="PSUM"),
                tc.tile_pool(name="ot_psum", bufs=2, space="PSUM"),
            )
            (pexp_pool, dvp, kts_pool, vtm_pool, dsum_pool, fout_pool,
             st_psum, ot_psum) = (p.__enter__() for p in attn_pools)

            # V path, two heads ahead: projection (PE, st-tag slot), fused
            # bias+mask (DVE, one scalar_tensor_tensor), DMA-transpose to
            # the natural [j, c] layout
            def emit_vpath(h):
                vp = st_psum.tile([128, N], F32, tag="st", name="vp")
                for half in range(2):
                    nc.tensor.matmul(
                        vp[:, half * 512 : (half + 1) * 512],
                        wv_sb,
                        xT_own[:, h, half * 512 : (half + 1) * 512],
                        start=True,
                        stop=True,
                    )
                vtm = vtm_pool.tile([128, N], BF16, name="vtm")
                nc.vector.scalar_tensor_tensor(
                    out=vtm, in0=vp, scalar=bv_sb, in1=mrow_b,
                    op0=Add, op1=Mult,
                )
                nc.sync.dma_start_transpose(out=vnat[:, h], in_=vtm)

            # kT scaled by the binary mask (free-dim multiply), emitted two
            # heads ahead of consumption. The first two go on DVE (free right
            # after the top-k chain) so head 0 starts ASAP; the rest on the
            # otherwise idle Pool engine.
            kts_tiles = {}

            def emit_kts(h, eng):
                kts = kts_pool.tile([128, N], BF16)
                eng.tensor_tensor(out=kts, in0=kT[:, h, :], in1=mrow_b, op=Mult)
                kts_tiles[h] = kts

            emit_kts(0, nc.vector)
            emit_kts(1, nc.gpsimd)
            emit_vpath(0)

            # flat (h, jt) pipeline: PV matmuls lag the exps by two steps and
            # flow across head boundaries, so the PE never waits on a head
            # transition. The softmax denominator is tree-summed pairwise in
            # bf16 as the exp tiles appear (DVE), partition-reduced +
            # broadcast on gpsimd, and applied with a fused
            # reciprocal-multiply normalize on DVE.
            heads = {}
            pv_queue = []

            def emit_pv(h, jt):
                st_h = heads[h]
                for half in range(2):
                    nc.tensor.matmul(
                        st_h["ot"][:, half * 512 : (half + 1) * 512],
                        vnat[:, h, jt, :],
                        st_h["pexp"][:, jt, half * 512 : (half + 1) * 512],
                        start=(jt == 0),
                        stop=(jt == 7),
                    )
                if jt == 7:
                    finish_head(h)

            def finish_head(h):
                hh = heads.pop(h)
                e01, e23, e45, e67 = hh["esum"]
                nc.vector.tensor_tensor(out=e45, in0=e45, in1=e67, op=Add)
                nc.vector.tensor_tensor(out=e01, in0=e01, in1=e23, op=Add)
                nc.vector.tensor_tensor(out=e01, in0=e01, in1=e45, op=Add)
                den = dvp.tile([128, N], F32)
                nc.gpsimd.partition_all_reduce(
                    den, e01, channels=128, reduce_op=bass_isa.ReduceOp.add
                )
                nc.vector.reciprocal(den, den)
                nc.vector.tensor_tensor(
                    out=outT_sb[:, h, :], in0=hh["ot"], in1=den, op=Mult
                )

            for h in range(HPC):
                if h + 2 < HPC:
                    emit_kts(h + 2, nc.gpsimd)
                if h + 1 < HPC:
                    emit_vpath(h + 1)
                kts_h = kts_tiles.pop(h)
                heads[h] = {
                    "ot": ot_psum.tile([128, N], F32, name="ot"),
                    "pexp": pexp_pool.tile([128, 8, N], BF16, name="pexp"),
                    "esum": [],
                }
                pexp = heads[h]["pexp"]
                for jt in range(8):
                    st = st_psum.tile([128, N], F32, tag="st")
                    for half in range(2):
                        nc.tensor.matmul(
                            st[:, half * 512 : (half + 1) * 512],
                            kts_h[:, jt * 128 : (jt + 1) * 128],
                            qT[:, h, half * 512 : (half + 1) * 512],
                            start=True,
                            stop=True,
                        )
                    nc.scalar.activation(
                        out=pexp[:, jt, :], in_=st, func=Exp, scale=SCALE,
                    )
                    if jt % 2 == 1:
                        es = dsum_pool.tile([128, N], BF16)
                        nc.vector.tensor_tensor(
                            out=es, in0=pexp[:, jt - 1, :],
                            in1=pexp[:, jt, :], op=Add,
                        )
                        heads[h]["esum"].append(es)
                    pv_queue.append((h, jt))
                    while len(pv_queue) > 2:
                        emit_pv(*pv_queue.pop(0))
            while pv_queue:
                emit_pv(*pv_queue.pop(0))

            # ============= phase 3: to_out partial =========================
            # fo shares the st_psum slots (same shape) so Wo accumulation can
            # begin as soon as the last exp frees an ST slot. Heads 1..7 are
            # accumulated in the per-oc loop; head 0 (whose outT is ready
            # first) finishes each oc one iteration later.
            def finish_oc(oc, fo):
                for half in range(2):
                    nc.tensor.matmul(
                        fo[:, half * 512 : (half + 1) * 512],
                        woT_sb[:, 0, oc * 128 : (oc + 1) * 128],
                        outT_sb[:, 0, half * 512 : (half + 1) * 512],
                        start=False,
                        stop=True,
                    )
                fout = fout_pool.tile([128, N], BF16)
                nc.scalar.activation(out=fout, in_=fo, func=Copy)
                for sh in range(2):
                    eng = nc.sync if sh == 0 else nc.scalar
                    eng.dma_start(
                        out=outT_d[oc * 128 : (oc + 1) * 128,
                                   sh * 512 : (sh + 1) * 512],
                        in_=fout[:, sh * 512 : (sh + 1) * 512],
                    )

            pending_oc = None
            for oc in range(16):
                fo = st_psum.tile([128, N], F32, tag="st")
                for h in range(1, HPC):
                    for half in range(2):
                        nc.tensor.matmul(
                            fo[:, half * 512 : (half + 1) * 512],
                            woT_sb[:, h, oc * 128 : (oc + 1) * 128],
                            outT_sb[:, h, half * 512 : (half + 1) * 512],
                            start=(h == 1),
                            stop=False,
                        )
                if pending_oc is not None:
                    finish_oc(*pending_oc)
                pending_oc = (oc, fo)
            finish_oc(*pending_oc)

            for p in reversed(attn_pools):
                p.__exit__(None, None, None)


_CACHE = {}


def _get_module():
    if "nc" in _CACHE:
        return _CACHE["nc"]
    nc = bacc.Bacc("TRN2", target_bir_lowering=False, debug=False, num_devices=8)
    xTd = nc.dram_tensor("xT", (D, N), BF16, kind="ExternalInput").ap()
    wq_d = nc.dram_tensor("wqT", (C, C), BF16, kind="ExternalInput").ap()
    wk_d = nc.dram_tensor("wkT", (C, C), BF16, kind="ExternalInput").ap()
    wv_d = nc.dram_tensor("wvT", (C, C), BF16, kind="ExternalInput").ap()
    bq_d = nc.dram_tensor("bq", (C, 1), F32, kind="ExternalInput").ap()
    bk_d = nc.dram_tensor("bk", (C, 1), F32, kind="ExternalInput").ap()
    bv_d = nc.dram_tensor("bv", (C, 1), F32, kind="ExternalInput").ap()
    wtc_d = nc.dram_tensor("wtc", (C, 1), BF16, kind="ExternalInput").ap()
    wo_d = nc.dram_tensor("woT", (HPC * C, D), BF16, kind="ExternalInput").ap()
    outT_d = nc.dram_tensor("outT", (D, N), BF16, kind="ExternalOutput").ap()

    with tile.TileContext(nc) as tc:
        _body(tc, xTd, wq_d, wk_d, wv_d, bq_d, bk_d, bv_d, wtc_d, wo_d, outT_d)
    nc.compile()
    _CACHE["nc"] = nc
    return nc


def make_in_maps(x, Wq, bq, Wk, bk, Wv, bv, Wl, bl, Wo, bo):
    x = np.asarray(x, np.float32)
    Wq = np.asarray(Wq, np.float32)
    Wk = np.asarray(Wk, np.float32)
    Wv = np.asarray(Wv, np.float32)
    Wl = np.asarray(Wl, np.float32)
    Wo = np.asarray(Wo, np.float32)
    we = (Wl[0] @ Wq) / float(NCHUNK)  # (128,) logits weight per chunk
    bf = ml_dtypes.bfloat16
    common = {
        "wqT": np.ascontiguousarray(Wq.T).astype(bf),
        "wkT": np.ascontiguousarray(Wk.T).astype(bf),
        "wvT": np.ascontiguousarray(Wv.T).astype(bf),
        "bq": np.asarray(bq, np.float32).reshape(C, 1),
        "bk": np.asarray(bk, np.float32).reshape(C, 1),
        "bv": np.asarray(bv, np.float32).reshape(C, 1),
        "wtc": we.reshape(C, 1).astype(bf),
    }
    woT = np.ascontiguousarray(Wo.T)  # (d, o)
    woT_half = [
        woT[0:1024, :].astype(bf),
        woT[1024:2048, :].astype(bf),
    ]
    in_maps = []
    xT_whole = [np.ascontiguousarray(x[b].T).astype(bf) for b in range(B)]
    for core in range(8):
        b, g = divmod(core, 2)
        xbT = xT_whole[b]
        xcore = xbT if g == 0 else np.ascontiguousarray(
            np.concatenate([xbT[1024:], xbT[:1024]], axis=0)
        )
        in_maps.append({"xT": xcore, "woT": woT_half[g], **common})
    return in_maps


def run_spmd(in_maps, trace=False, **kw):
    nc = _get_module()
    return bass_utils.run_bass_kernel_spmd(
        nc, in_maps, core_ids=list(range(8)), trace=trace, **kw
    )


def gather(results, bo):
    bo = np.asarray(bo, np.float32)
    out = np.empty((B, N, D), np.float32)
    for b in range(B):
        p0 = results[2 * b]["outT"].astype(np.float32).T
        p1 = results[2 * b + 1]["outT"].astype(np.float32).T
        out[b] = p0 + p1 + bo
    return out


def kernel(x, Wq, bq, Wk, bk, Wv, bv, Wl, bl, Wo, bo, stage=None, **_unused):
    in_maps = make_in_maps(x, Wq, bq, Wk, bk, Wv, bv, Wl, bl, Wo, bo)
    try:
        res = run_spmd(in_maps)
    except Exception:
        # transient device/runtime hiccup: retry once after a short pause
        import time as _time

        _time.sleep(2.0)
        res = run_spmd(in_maps)
    return gather(res.results, bo)
